# revision 1
# baseline (speedup 1.0000x reference)
"""Trainium2 Bass kernel for nn_ModalityPooling (segment attention-pooling).

Strategy (8 NeuronCores, SPMD):
  - Shard nodes of each modality into 8 contiguous ranges (data parallel);
    per-core ranges are padded with zero rows / batch-id 64 to a multiple of
    the 512-node macro tile so all cores run one identical program.
  - Single streaming pass over node features x (the memory-bound bulk):
    per 128-node subtile compute scorer s = relu(x@w1+b1)@w2+b2 on the
    tensor engine (via PE transpose of x), e = exp(s) (no max subtraction
    needed: softmax weights e/sum(e) are invariant and |s| is tiny for this
    distribution), build masked weights W[n, b] = (batch[n]==b) * e[n] with
    one dual-op tensor_scalar against an iota tile, and accumulate
    num[b, :] += W.T @ [x | 1] into a persistent PSUM bank. Column 256 of
    num is the softmax denominator.
  - Each core emits raw partials (64, 257) per modality; the host sums the
    8 partials (the cross-core "unshard"), normalizes pooled = num/den and
    applies the tiny (64x256) MLP heads in float32 numpy.
"""

import sys

import numpy as np

if "/opt/trn_rl_repo" not in sys.path:
    sys.path.insert(0, "/opt/trn_rl_repo")

import concourse.bass as bass  # noqa: E402
import concourse.mybir as mybir  # noqa: E402
import concourse.tile as tile  # noqa: E402
from concourse import bacc  # noqa: E402
from concourse import bass_utils  # noqa: E402

F32 = mybir.dt.float32

NCORES = 8
D = 256
H = 64
B = 64
P = 128
MT = 512  # nodes per macro tile
SUB = MT // P  # 128-node subtiles per macro tile

# (tag, total nodes) per modality; shapes are hardcoded per the task contract.
MODS = [("g", 400000), ("c", 600000), ("r", 50000)]


def _per_core_padded(n_total: int) -> tuple[int, int]:
    per = n_total // NCORES
    assert per * NCORES == n_total
    padded = ((per + MT - 1) // MT) * MT
    return per, padded


def build_program(trace_annotate: bool = False):
    """Build the SPMD Bass program (identical for all 8 cores)."""
    nc = bacc.Bacc("TRN2", target_bir_lowering=False, debug=False,
                   num_devices=NCORES)

    dram_in = {}
    dram_out = {}
    # Constants shared by all modalities
    dram_in["ident"] = nc.dram_tensor("ident", [P, P], F32, kind="ExternalInput")
    dram_in["iota"] = nc.dram_tensor("iota", [P, H], F32, kind="ExternalInput")
    for tag, n_total in MODS:
        _, npad = _per_core_padded(n_total)
        dram_in[f"x_{tag}"] = nc.dram_tensor(f"x_{tag}", [npad, D], F32,
                                             kind="ExternalInput")
        dram_in[f"bf_{tag}"] = nc.dram_tensor(f"bf_{tag}", [npad], F32,
                                              kind="ExternalInput")
        dram_in[f"w1_{tag}"] = nc.dram_tensor(f"w1_{tag}", [D, H], F32,
                                              kind="ExternalInput")
        dram_in[f"b1_{tag}"] = nc.dram_tensor(f"b1_{tag}", [H, 1], F32,
                                              kind="ExternalInput")
        dram_in[f"w2_{tag}"] = nc.dram_tensor(f"w2_{tag}", [H, 1], F32,
                                              kind="ExternalInput")
        dram_in[f"b2_{tag}"] = nc.dram_tensor(f"b2_{tag}", [P, 1], F32,
                                              kind="ExternalInput")
        dram_out[f"out_{tag}"] = nc.dram_tensor(f"out_{tag}", [B, D + 1], F32,
                                                kind="ExternalOutput")

    with tile.TileContext(nc) as tc:
        with (
            tc.tile_pool(name="const", bufs=1) as cpool,
            tc.tile_pool(name="data", bufs=3) as dpool,
            tc.tile_pool(name="work", bufs=3) as wpool,
            tc.tile_pool(name="psum", bufs=2, space="PSUM") as ppool,
            tc.tile_pool(name="acc", bufs=1, space="PSUM") as apool,
        ):
            ident = cpool.tile([P, P], F32)
            nc.sync.dma_start(ident[:], dram_in["ident"].ap())
            iota = cpool.tile([P, H], F32)
            nc.sync.dma_start(iota[:], dram_in["iota"].ap())

            for tag, n_total in MODS:
                _, npad = _per_core_padded(n_total)
                nmac = npad // MT

                # Params
                w1_sb = cpool.tile([P, D // P, H], F32, name=f"w1sb_{tag}")
                nc.sync.dma_start(
                    w1_sb[:],
                    dram_in[f"w1_{tag}"].ap().rearrange("(c p) j -> p c j", p=P),
                )
                b1_sb = cpool.tile([H, 1], F32, name=f"b1sb_{tag}")
                nc.sync.dma_start(b1_sb[:], dram_in[f"b1_{tag}"].ap())
                w2_sb = cpool.tile([H, 1], F32, name=f"w2sb_{tag}")
                nc.sync.dma_start(w2_sb[:], dram_in[f"w2_{tag}"].ap())
                b2_sb = cpool.tile([P, 1], F32, name=f"b2sb_{tag}")
                nc.sync.dma_start(b2_sb[:], dram_in[f"b2_{tag}"].ap())

                # Whole (padded) batch-id array as f32, one column per subtile
                ncols = npad // P
                bf_sb = cpool.tile([P, ncols], F32, name=f"bfsb_{tag}")
                nc.sync.dma_start(
                    bf_sb[:], dram_in[f"bf_{tag}"].ap().rearrange("(t p) -> p t", p=P)
                )

                x_r = dram_in[f"x_{tag}"].ap().rearrange(
                    "(t j p) f -> t p j f", p=P, j=SUB
                )

                num_ps = apool.tile([B, D + 1], F32, tag="num", name=f"num_{tag}")

                for t in range(nmac):
                    # Load 512 nodes; column 256 of each subtile row is 1.0
                    xe = dpool.tile([P, SUB, D + 1], F32, tag="xe", name="xe")
                    nc.sync.dma_start(xe[:, :, 0:D], x_r[t])
                    nc.vector.memset(xe[:, :, D:D + 1], 1.0)

                    # Transpose x -> xt (feature-major) via PE
                    xt = wpool.tile([P, D // P, MT], F32, tag="xt", name="xt")
                    for c in range(D // P):
                        xt_ps = ppool.tile([P, MT], F32, tag="xt_ps", name="xt_ps")
                        for j in range(SUB):
                            nc.tensor.transpose(
                                xt_ps[:, j * P:(j + 1) * P],
                                xe[:, j, c * P:(c + 1) * P],
                                ident,
                            )
                        nc.scalar.copy(xt[:, c, :], xt_ps[:])

                    # h^T = (x @ w1)^T : accumulate over the two 128-feat chunks
                    h_ps = ppool.tile([H, MT], F32, tag="h_ps", name="h_ps")
                    for c in range(D // P):
                        nc.tensor.matmul(h_ps[:], w1_sb[:, c, :], xt[:, c, :],
                                         start=(c == 0), stop=(c == D // P - 1))
                    hr = wpool.tile([H, MT], F32, tag="hr", name="hr")
                    nc.scalar.activation(hr[:], h_ps[:],
                                         mybir.ActivationFunctionType.Relu,
                                         bias=b1_sb[:], scale=1.0)

                    # s (node-major): s[:, j] = hr_j^T @ w2   (128 nodes x 1)
                    s_ps = ppool.tile([P, SUB], F32, tag="s_ps", name="s_ps",
                                      bufs=1)
                    for j in range(SUB):
                        nc.tensor.matmul(s_ps[:, j:j + 1],
                                         hr[:, j * P:(j + 1) * P], w2_sb[:],
                                         start=True, stop=True)
                    e_sb = wpool.tile([P, SUB], F32, tag="e_sb", name="e_sb")
                    nc.scalar.activation(e_sb[:], s_ps[:],
                                         mybir.ActivationFunctionType.Exp,
                                         bias=b2_sb[:], scale=1.0)

                    # W[n, b] = (iota[b] == batch[n]) * e[n]
                    w_sb = wpool.tile([P, SUB, H], F32, tag="w_sb", name="w_sb")
                    for j in range(SUB):
                        col = t * SUB + j
                        nc.vector.tensor_scalar(
                            out=w_sb[:, j, :],
                            in0=iota[:],
                            scalar1=bf_sb[:, col:col + 1],
                            scalar2=e_sb[:, j:j + 1],
                            op0=mybir.AluOpType.is_equal,
                            op1=mybir.AluOpType.mult,
                        )

                    # num += W^T @ [x | 1]
                    for j in range(SUB):
                        first = (t == 0 and j == 0)
                        last = (t == nmac - 1 and j == SUB - 1)
                        nc.tensor.matmul(num_ps[:], w_sb[:, j, :], xe[:, j, :],
                                         start=first, stop=last)

                out_sb = wpool.tile([B, D + 1], F32, tag="out_sb",
                                    name=f"outsb_{tag}")
                nc.scalar.copy(out_sb[:], num_ps[:])
                nc.sync.dma_start(dram_out[f"out_{tag}"].ap(), out_sb[:])

    nc.compile()
    return nc


def _prep_core_inputs(x_gene, x_cpg, x_mir, batch_gene, batch_cpg, batch_mir,
                      params):
    """Build the 8 per-core input maps (host-side shard + pad)."""
    ident = np.eye(P, dtype=np.float32)
    iota = np.broadcast_to(np.arange(H, dtype=np.float32), (P, H)).copy()

    mod_data = {
        "g": (x_gene, batch_gene, params["pool_gene"]),
        "c": (x_cpg, batch_cpg, params["pool_cpg"]),
        "r": (x_mir, batch_mir, params["pool_mir"]),
    }

    common = {"ident": ident, "iota": iota}
    for tag, n_total in MODS:
        _, _, p = mod_data[tag]
        w1, b1 = p["w1b1"]
        w2, b2 = p["w2b2"]
        common[f"w1_{tag}"] = np.ascontiguousarray(w1, dtype=np.float32)
        common[f"b1_{tag}"] = np.ascontiguousarray(
            np.asarray(b1, dtype=np.float32).reshape(H, 1))
        common[f"w2_{tag}"] = np.ascontiguousarray(w2, dtype=np.float32).reshape(H, 1)
        common[f"b2_{tag}"] = np.full((P, 1), np.float32(np.asarray(b2).reshape(-1)[0]),
                                      dtype=np.float32)

    in_maps = []
    for c in range(NCORES):
        m = dict(common)
        for tag, n_total in MODS:
            x, batch, _ = mod_data[tag]
            per, npad = _per_core_padded(n_total)
            st, en = c * per, (c + 1) * per
            xp = np.zeros((npad, D), dtype=np.float32)
            xp[:per] = np.asarray(x[st:en], dtype=np.float32)
            bf = np.full((npad,), np.float32(B), dtype=np.float32)
            bf[:per] = np.asarray(batch[st:en]).astype(np.float32)
            m[f"x_{tag}"] = xp
            m[f"bf_{tag}"] = bf
        in_maps.append(m)
    return in_maps


def _finish_on_host(sums, params):
    """Normalize pooled vectors and apply the small linear heads (f32 numpy)."""
    pooled = {}
    for tag, _ in MODS:
        num = sums[tag]
        den = num[:, D:D + 1]
        with np.errstate(divide="ignore", invalid="ignore"):
            pool = np.where(den > 0, num[:, :D] / den, 0.0).astype(np.float32)
        pooled[tag] = pool

    def mlp_head(v, p):
        w1, b1 = p["w1b1"]
        w2, b2 = p["w2b2"]
        w1 = np.asarray(w1, np.float32)
        b1 = np.asarray(b1, np.float32)
        w2 = np.asarray(w2, np.float32)
        b2 = np.asarray(b2, np.float32)
        return np.maximum(v @ w1 + b1, 0.0) @ w2 + b2

    z_mrna = mlp_head(pooled["g"], params["mrna"]).astype(np.float32)
    z_cnv = mlp_head(pooled["g"], params["cnv"]).astype(np.float32)
    wc, bc = params["lin_cpg"]
    wm, bm = params["lin_mir"]
    z_dnam = (pooled["c"] @ np.asarray(wc, np.float32)
              + np.asarray(bc, np.float32)).astype(np.float32)
    z_mir = (pooled["r"] @ np.asarray(wm, np.float32)
             + np.asarray(bm, np.float32)).astype(np.float32)
    return (z_mrna, z_cnv, z_dnam, z_mir)


_PROGRAM_CACHE = {}


def run(x_gene, x_cpg, x_mir, batch_gene, batch_cpg, batch_mir, params,
        trace=False, trace_cores=None):
    """Run on 8 NeuronCores; returns (outputs_tuple, BassKernelResults)."""
    if "nc" not in _PROGRAM_CACHE:
        _PROGRAM_CACHE["nc"] = build_program()
    nc = _PROGRAM_CACHE["nc"]

    in_maps = _prep_core_inputs(x_gene, x_cpg, x_mir, batch_gene, batch_cpg,
                                batch_mir, params)
    kwargs = {}
    if trace:
        kwargs["trace"] = True
        if trace_cores is not None:
            kwargs["trace_cores"] = trace_cores
    res = bass_utils.run_bass_kernel_spmd(
        nc, in_maps, core_ids=list(range(NCORES)), **kwargs
    )

    sums = {}
    for tag, _ in MODS:
        acc = np.zeros((B, D + 1), dtype=np.float64)
        for c in range(NCORES):
            acc += res.results[c][f"out_{tag}"].astype(np.float64)
        sums[tag] = acc.astype(np.float32)

    out = _finish_on_host(sums, params)
    return out, res


def kernel(x_gene, x_cpg, x_mir, batch_gene, batch_cpg, batch_mir, params):
    out, _ = run(x_gene, x_cpg, x_mir, batch_gene, batch_cpg, batch_mir, params)
    return out
